# revision 27
# baseline (speedup 1.0000x reference)
"""Bass/Trainium2 kernel for conv-QKV multi-head attention.

Problem: x (2,5,640,32,32); 3x3 SAME conv projections Q/K/V (640->640);
8-head attention over N=1024 tokens per (b,m) crop, head_dim=80; output
projection (640x640) applied per (b,n,m); output (2,1024,3200).

Sharding: tensor-parallel by head. Core h computes the 240 conv output
channels for head h's q/k/v (channel order [q,k,v], two PSUM tiles of
128+112 rows), full attention for its head over all 10 crops, and a
partial output projection against w_proj[:, h*80:(h+1)*80]. The 8
partial outputs are summed on the host.

Conv: 9 shifted matmuls (SAME padding via a zero-padded 34x34 SBUF
image, the x-window is the *moving* operand so its 2-D access pattern
is legal) accumulated in PSUM over 5 input-channel tiles. Matmul
operands must start at partition 0/32/64 with equal bases, so K is
rebased to partition 0 with SBUF->SBUF DMAs and V (living at base 32)
is transposed against a diagonal-at-offset-32 matrix. Compute is bf16
on TensorE with f32 PSUM accumulation.
"""

import numpy as np
import ml_dtypes
from contextlib import ExitStack

BS, MC, C, H, W = 2, 5, 640, 32, 32
NH, HD = 8, 80
N = H * W           # 1024
CROPS = BS * MC     # 10
CIT = C // 128      # 5 input-channel tiles
PH, PW = H + 2, W + 2   # 34x34 padded image
SCALE = HD ** -0.5
NCORES = 8

_BF16 = ml_dtypes.bfloat16


def _build_graph():
    import concourse.bacc as bacc
    from concourse import bass, mybir, tile, masks

    f32 = mybir.dt.float32
    bf16 = mybir.dt.bfloat16
    X = mybir.AxisListType.X
    Exp = mybir.ActivationFunctionType.Exp
    Ident = mybir.ActivationFunctionType.Identity

    nc = bacc.Bacc("TRN2", target_bir_lowering=False, debug=False,
                   num_devices=NCORES)

    x_ext = nc.declare_dram_parameter("x", [BS, MC, C, H, W], f32, isOutput=False)
    wqkv_ext = nc.declare_dram_parameter("wqkv", [CIT, 128, 9 * 256], bf16, isOutput=False)
    bqkv_ext = nc.declare_dram_parameter("bqkv", [256, 1], f32, isOutput=False)
    wproj_ext = nc.declare_dram_parameter("wproj", [HD, C], bf16, isOutput=False)
    out_ext = nc.declare_dram_parameter("out", [CROPS, 641, N], f32, isOutput=True)

    with tile.TileContext(nc) as tc, ExitStack() as ctx:
        const = ctx.enter_context(tc.tile_pool(name="const", bufs=1))
        sb = ctx.enter_context(tc.tile_pool(name="sb", bufs=2))
        psum = ctx.enter_context(tc.tile_pool(name="psum", bufs=2, space="PSUM"))

        ident = const.tile([128, 128], bf16, tag="ident")
        masks.make_identity(nc, ident[:])

        w_sb = []
        for t in range(CIT):
            wt = const.tile([128, 9 * 256], bf16, tag=f"wqkv{t}", name=f"wqkv{t}")
            nc.sync.dma_start(wt[:], wqkv_ext[t])
            w_sb.append(wt)
        bias_a = const.tile([128, 1], f32, tag="bias_a")
        nc.sync.dma_start(bias_a[:], bqkv_ext[0:128])
        bias_b = const.tile([128, 1], f32, tag="bias_b")
        nc.sync.dma_start(bias_b[:], bqkv_ext[128:256])
        wp_sb = const.tile([HD, C], bf16, tag="wproj")
        nc.sync.dma_start(wp_sb[:], wproj_ext[:])

        # Double-buffered (across crops) persistent tiles.
        NXB = 3   # triple-buffer the padded input (prefetch depth 2)
        xpadf = [[const.tile([128, PH * PW], f32, tag=f"xf{s}_{t}", name=f"xf{s}_{t}")
                  for t in range(CIT)] for s in range(NXB)]
        xpadb = [[const.tile([128, PH * PW], bf16, tag=f"xb{s}_{t}", name=f"xb{s}_{t}")
                  for t in range(CIT)] for s in range(NXB)]
        for s in range(NXB):
            for t in range(CIT):
                nc.vector.memset(xpadf[s][t][:], 0.0)
        pT = [[const.tile([128, N], bf16, tag=f"pT{s}_{kb}", name=f"pT{s}_{kb}")
               for kb in range(8)] for s in range(2)]
        # vT blocks of 97 columns: [0:80] = v^T, [80:96] zero, col 96 = ones
        # (the O matmul then also emits the softmax row-sum as output row 96,
        # a 32-aligned partition, which DVE can read).
        VB = 97
        vT2 = [const.tile([128, 8 * VB], bf16, tag=f"vt{s}", name=f"vt{s}")
               for s in range(2)]
        for s in range(2):
            nc.vector.memset(vT2[s][:], 0.0)
            for kb in range(8):
                nc.vector.memset(vT2[s][:, kb * VB + 96: kb * VB + VB], 1.0)

        def xload(c):
            """DMA crop c into padded f32 image, cast to bf16."""
            sx = c % NXB
            b, m = divmod(c, MC)
            for t in range(CIT):
                xv = xpadf[sx][t][:].rearrange("p (h w) -> p h w", h=PH)
                nc.sync.dma_start(xv[:, 1:1 + H, 1:1 + W],
                                  x_ext[b, m, t * 128:(t + 1) * 128])
                nc.vector.tensor_copy(xpadb[sx][t][:], xpadf[sx][t][:])

        qkv_of = {}

        def conv_phase(c, fillers=()):
            """Conv QKV: A = [q80, v0:48]; B = [k80, v48:80, 16 dead].
            `fillers` are PE work units (previous crop's S^T/vT) spliced
            between conv weight-groups so their ACT/DVE consumers overlap
            the conv stream."""
            sx = c % NXB
            qkv_a = sb.tile([128, N], bf16, tag="qkv_a")
            qkv_b = sb.tile([128, N], bf16, tag="qkv_b")
            qkv_of[c] = (qkv_a, qkv_b)
            fillers = list(fillers)
            ngroups = 90
            fill_every = max(1, ngroups // (len(fillers) + 1)) if fillers else 0
            gi = 0
            for co in range(2):
                pc = psum.tile([128, N], f32, tag="mm")
                ki = 0
                for dy in range(3):
                    for dx in range(3):
                        off = dy * 3 + dx
                        for t in range(CIT):
                            xb = xpadb[sx][t][:].rearrange(
                                "p (h w) -> p h w", h=PH)
                            lhsT = w_sb[t][:, off * 256 + co * 128:
                                           off * 256 + co * 128 + 128]
                            for h2 in range(2):
                                rhs = xb[:, dy + h2 * 16: dy + h2 * 16 + 16,
                                         dx: dx + W]
                                nc.tensor.matmul(
                                    pc[:, h2 * 512:(h2 + 1) * 512], lhsT, rhs,
                                    start=(ki == 0), stop=(ki == 44))
                            ki += 1
                            gi += 1
                            if (fillers and gi % fill_every == 0
                                    and gi // fill_every <= len(fillers)):
                                fillers[gi // fill_every - 1]()
                if co == 0:
                    nc.scalar.activation(qkv_a[:], pc[:], Ident, bias=bias_a[:])
                else:
                    nc.scalar.activation(qkv_b[:], pc[:], Ident, bias=bias_b[:])

        ot_of = {}

        def attn_fillers(c):
            """Per-kb PE units of crop c's attention front half: vT
            transpose + S^T matmuls (+ACT exp into pT)."""
            s = c % 2
            qkv_a, qkv_b = qkv_of[c]
            v_sb = sb.tile([HD, N], bf16, tag="v_sb")
            nc.sync.dma_start(v_sb[0:48, :], qkv_a[80:128, :])
            nc.sync.dma_start(v_sb[48:80, :], qkv_b[80:112, :])
            vt = vT2[s]

            def unit(kb):
                def f():
                    tv = psum.tile([128, HD], bf16, tag="st", bufs=2)
                    nc.tensor.transpose(tv[:], v_sb[:, kb * 128:(kb + 1) * 128],
                                        ident[0:HD, 0:HD])
                    nc.vector.tensor_copy(vt[:, kb * VB:kb * VB + HD], tv[:])
                    st = psum.tile([128, N], f32, tag="st", bufs=2)
                    for h2 in range(2):
                        nc.tensor.matmul(
                            st[:, h2 * 512:(h2 + 1) * 512],
                            qkv_b[0:HD, kb * 128:(kb + 1) * 128],
                            qkv_a[0:HD, h2 * 512:(h2 + 1) * 512],
                            start=True, stop=True)
                    nc.scalar.activation(pT[s][kb][:], st[:], Exp, scale=SCALE)
                return f

            return [unit(kb) for kb in range(8)]

        def attn_tail(c):
            """O matmul + output staging for crop c (after its fillers)."""
            s = c % 2
            qkv_of.pop(c)
            vt = vT2[s]
            # [O^T; rowsum] = [V^T; 1]^T P^T  (row 96 = softmax sums)
            po = psum.tile([VB, N], f32, tag="mm")
            for kb in range(8):
                for h2 in range(2):
                    nc.tensor.matmul(
                        po[:, h2 * 512:(h2 + 1) * 512],
                        vt[:, kb * VB:(kb + 1) * VB],
                        pT[s][kb][:, h2 * 512:(h2 + 1) * 512],
                        start=(kb == 0), stop=(kb == 7))
            # Normalization is deferred to the host: ship row-sums, copy
            # unnormalized O^T straight to the projection input.
            ot = sb.tile([HD, N], bf16, tag="ot")
            nc.vector.tensor_copy(ot[:], po[0:HD, :])
            ot_of[c] = ot
            rrow = sb.tile([1, N], f32, tag="rrow")
            nc.vector.tensor_copy(rrow[:], po[96:97, :])
            nc.sync.dma_start(out_ext[c, 640:641, :], rrow[:])

        def proj_phase(c, fillers=()):
            ot = ot_of.pop(c)
            fillers = list(fillers)
            for dt in range(5):
                pp = psum.tile([128, N], f32, tag="mm")
                for h2 in range(2):
                    nc.tensor.matmul(
                        pp[:, h2 * 512:(h2 + 1) * 512],
                        wp_sb[:, dt * 128:(dt + 1) * 128],
                        ot[:, h2 * 512:(h2 + 1) * 512],
                        start=True, stop=True)
                osb = sb.tile([128, N], f32, tag="osb")
                if dt % 2 == 0:
                    nc.scalar.activation(osb[:], pp[:], Ident)
                else:
                    nc.vector.tensor_copy(osb[:], pp[:])
                nc.sync.dma_start(out_ext[c, dt * 128:(dt + 1) * 128, :], osb[:])
                for _ in range(2):
                    if fillers:
                        fillers.pop(0)()
            for f in fillers:
                f()

        # Software-pipelined emission: crop c's S^T/vT units are spliced
        # into crop c+1's conv stream (their ACT exp overlaps conv), then
        # O(c) and proj(c-1) follow. PE never idles at phase boundaries.
        xload(0)
        xload(1)
        # PE warmup: dummy matmuls on the identity while the first crop
        # loads, so conv(0) starts at full clock.
        warm = psum.tile([128, 128], f32, tag="st", bufs=2)
        for _ in range(30):
            nc.tensor.matmul(warm[:], ident[:], ident[:], start=True, stop=True)
        conv_phase(0)
        for c in range(CROPS):
            if c + 2 < CROPS:
                xload(c + 2)
            fills = attn_fillers(c)
            if c + 1 < CROPS:
                conv_phase(c + 1, fills)
                attn_tail(c)
                if c >= 1:
                    proj_phase(c - 1)
            else:
                # epilogue: last crop's S^T/vT units hide inside proj(c-1)
                proj_phase(c - 1, fills)
                attn_tail(c)
        proj_phase(CROPS - 1)

    nc.compile()
    return nc


def _host_inputs(x, wq, bq, wk, bk, wv, bv, w_proj):
    """Per-core input maps; conv output channels ordered [q, k, v]."""
    in_maps = []
    x32 = np.ascontiguousarray(x, dtype=np.float32)
    for h in range(NCORES):
        sl = slice(h * HD, (h + 1) * HD)
        zpad = np.zeros((16,) + wq.shape[1:], wq.dtype)
        w_cat = np.concatenate(
            [wq[sl], wv[sl][:48], wk[sl], wv[sl][48:], zpad], axis=0)  # [256,...]
        # [co, ci, dy, dx] -> [ci, off, co] -> [5, 128, 9*256]
        wt = w_cat.transpose(2, 3, 1, 0).reshape(9, C, 256)
        wt = np.ascontiguousarray(wt.transpose(1, 0, 2)).reshape(CIT, 128, 9 * 256)
        b_cat = np.concatenate(
            [bq[sl], bv[sl][:48], bk[sl], bv[sl][48:],
             np.zeros(16, bq.dtype)]).reshape(256, 1)
        wpT = np.ascontiguousarray(w_proj[:, sl].T)  # [80, 640]
        in_maps.append({
            "x": x32,
            "wqkv": wt.astype(_BF16),
            "bqkv": b_cat.astype(np.float32),
            "wproj": wpT.astype(_BF16),
        })
    return in_maps


def _host_reduce(results, b_proj):
    acc = np.zeros((CROPS, C, N), np.float32)
    for r in results:
        acc += r["out"][:, :C, :] / r["out"][:, C:C + 1, :]
    o = acc.reshape(BS, MC, C, N).transpose(0, 3, 1, 2)  # [b, n, m, dout]
    o = o + b_proj[None, None, None, :].astype(np.float32)
    return np.ascontiguousarray(o.reshape(BS, N, MC * C), dtype=np.float32)


_NC_CACHE = {}


def kernel(x, wq, bq, wk, bk, wv, bv, w_proj, b_proj, _run_kwargs=None):
    from concourse.bass_utils import run_bass_kernel_spmd

    if "nc" not in _NC_CACHE:
        _NC_CACHE["nc"] = _build_graph()
    nc = _NC_CACHE["nc"]
    in_maps = _host_inputs(x, wq, bq, wk, bk, wv, bv, w_proj)
    res = run_bass_kernel_spmd(nc, in_maps, core_ids=list(range(NCORES)),
                               **(_run_kwargs or {}))
    out = _host_reduce(res.results, np.asarray(b_proj))
    if _run_kwargs:
        _NC_CACHE["last_result"] = res
    return out
